# revision 1
# baseline (speedup 1.0000x reference)
"""Trainium2 Bass kernel for nn_MixtureAttention.

Math: the reference builds a (c,c) pairwise Cauchy-product matrix per batch,
row-normalizes it, and keeps only the diagonal.  With
    u_d[c,p] = (mu[p,d] - mu[c,d]) / sig[c,d]
the kept diagonal reduces to
    coef[c] = 1 / sum_p prod_d 1/(1 + u_d[c,p]^2)
(`pi` cancels in the row normalization), and y[b,ch,c] = x[b,ch] * coef[b,c].

Sharding: 8 cores; core k handles batch k//2, c-rows [ (k%2)*2048, +2048 ).
Each core computes its 2048x4096 pairwise block fully on-chip, per
(128-row, 2048-point) tile:
  - ACT: 4x u_d^2 via Square activation with per-partition scale/bias
  - DVE: product chain [custom (a+1)(b+1) op, 2x affine_mul_reduce with the
    +1 folded into the bias slot, fast ~51-ULP reciprocal, tensor_scalar
    pass at fp32-2x whose accum_out carries the row-sum]
  - PE: final outer product x (x) coef, warmed before each epilogue half;
    the epilogue runs in two halves overlapped with the main loop
"""

import numpy as np

B, C, D, CH = 4, 4096, 4, 256
NCORES = 8
CW = C // 2            # 2048 c-rows per core (2 cores per batch)
NBLK = CW // 128       # 16 row blocks
PCH = 2048             # p-chunk size
NPCH = C // PCH        # 2
NOUT = 512             # matmul free-dim tile for the output outer product

_cache = {}


def _get_pp1():
    """Register a custom DVE op: out = (in0 + s0) * (in1 + s1).

    Fuses the '+1' pre-add into the pair product, saving one DVE pass per
    tile. Registered into concourse's op table at runtime; uop shas are
    self-pinned by compiling once and reading the reported digest.
    """
    if "pp1" in _cache:
        return _cache["pp1"]
    import re

    from concourse import dve_ops as DO
    from concourse.dve_spec import C0, C1, Spec, Src0, Src1

    name = "PROD_PLUS1_ANT"
    spec = Spec(
        body=(Src0 + C0) * (Src1 + C1),
        reference=lambda in0, in1, c0, c1, c2: (in0 + c0) * (in1 + c1),
    )
    shas = {}
    for ver in ("v3", "v4"):
        probe = DO.DveOp(name + "_PROBE", spec, subdim=False, uops_sha={})
        if name + "_PROBE" not in DO._SUB_OPCODE_FOR_NAME:
            DO._SUB_OPCODE_FOR_NAME[name + "_PROBE"] = 0x1F
        try:
            probe.compile(ver)
        except ValueError as e:
            m = re.search(r'"(?:v3|v4)"\]="([0-9a-f]+)"', str(e))
            if not m:
                raise
            shas[ver] = m.group(1)
    op = DO.DveOp(name, spec, subdim=False, uops_sha=shas)
    if name not in DO._SUB_OPCODE_FOR_NAME:
        DO.OPS.append(op)
        DO._SUB_OPCODE_FOR_NAME[name] = DO._CUSTOM_DVE_ROW_BASE + len(DO.OPS) - 1
        assert DO._SUB_OPCODE_FOR_NAME[name] < 0x20
    DO.CUSTOM_DVE_SPECS[name] = spec
    _cache["pp1"] = op
    return op


def _build(bench_nrep=None, bench_span="main"):
    import concourse.bacc as bacc
    import concourse.mybir as mybir
    from concourse.tile import TileContext

    f32 = mybir.dt.float32
    Alu = mybir.AluOpType
    Act = mybir.ActivationFunctionType

    pp1 = _get_pp1()
    nc = bacc.Bacc(None, target_bir_lowering=False)
    ptsT = nc.declare_dram_parameter("ptsT", [D, C], f32, isOutput=False)
    isg_r = nc.declare_dram_parameter("isg_r", [128, NBLK * D], f32, isOutput=False)
    nbs_r = nc.declare_dram_parameter("nbs_r", [128, NBLK * D], f32, isOutput=False)
    ps2_r = nc.declare_dram_parameter("ps2_r", [128, NBLK], f32, isOutput=False)
    xv = nc.declare_dram_parameter("xv", [1, CH], f32, isOutput=False)
    y = nc.declare_dram_parameter("y", [CH, CW], f32, isOutput=True)

    with TileContext(nc) as tc:
        with (
            tc.tile_pool(name="persist", bufs=1) as pp,
            tc.tile_pool(name="bpool", bufs=1) as bp,
            tc.tile_pool(name="work", bufs=1) as wp,
            tc.tile_pool(name="psum", bufs=4, space="PSUM") as psp,
            tc.tile_pool(name="dram", bufs=1, space="DRAM") as dp,
        ):
            scr = dp.tile([128 * NBLK], f32, name="scr")
            inv_sg = pp.tile([128, NBLK, D], f32)
            nc.sync.dma_start(
                out=inv_sg[:, :, :], in_=isg_r.rearrange("p (n d) -> p n d", d=D)
            )
            nbias = pp.tile([128, NBLK, D], f32)
            nc.sync.dma_start(
                out=nbias[:, :, :], in_=nbs_r.rearrange("p (n d) -> p n d", d=D)
            )
            ps2_sb = pp.tile([128, NBLK], f32)
            nc.sync.dma_start(out=ps2_sb[:, :], in_=ps2_r[:, :])
            xv_sb = pp.tile([1, CH], f32)
            nc.sync.dma_start(out=xv_sb[0:1, :], in_=xv[0:1, :])

            Racc = pp.tile([128, NBLK, NPCH], f32)
            junkacc = pp.tile([128, 2], f32)

            Bt = [bp.tile([128, C], f32, name=f"bt{dd}") for dd in range(D)]

            def bcast_loop():
                hp = PCH // 2
                for jj in range(2 * NPCH):
                    for dd in range(D):
                        nc.sync.dma_start(
                            out=Bt[dd][:, jj * hp : (jj + 1) * hp],
                            in_=ptsT[dd : dd + 1, jj * hp : (jj + 1) * hp].broadcast_to(
                                [128, hp]
                            ),
                        )

            def main_loop(n_lo, n_hi):
              for n in range(n_lo, n_hi):
                for j in range(NPCH):
                    sq = []
                    for dd in range(D):
                        s = wp.tile([128, PCH], f32, tag="sq", bufs=6, name="sq")
                        nc.scalar.activation(
                            s[:, :],
                            Bt[dd][:, j * PCH : (j + 1) * PCH],
                            Act.Square,
                            bias=nbias[:, n, dd : dd + 1],
                            scale=1.0,
                        )
                        sq.append(s)
                    # chain: Q = ((1+sq0)(1+sq1))(1+sq2))(1+sq3); first pair fused
                    q1 = wp.tile([128, PCH], f32, tag="q", bufs=4, name="q1")
                    nc.vector._custom_dve(
                        pp1, out=q1[:, :], in0=sq[0][:, :], in1=sq[1][:, :],
                        s0=inv_sg[:, n, 0:1], s1=inv_sg[:, n, 1:2],
                    )
                    q2 = wp.tile([128, PCH], f32, tag="q", bufs=4, name="q2")
                    nc.vector.affine_mul_reduce(
                        out=q2[:, :], accum_out=junkacc[:, 1:2],
                        in0=sq[2][:, :], in1=q1[:, :], scale=1.0,
                        bias=inv_sg[:, n, 2:3],
                    )
                    q3 = wp.tile([128, PCH], f32, tag="q", bufs=4, name="q3")
                    nc.vector.affine_mul_reduce(
                        out=q3[:, :], accum_out=junkacc[:, 0:1],
                        in0=sq[3][:, :], in1=q2[:, :], scale=1.0,
                        bias=inv_sg[:, n, 3:4],
                    )
                    # reciprocal + row-sum: every 4th iteration runs both fused
                    # on ACT (its Reciprocal table measures 1.2e-5 max rel err,
                    # fine for summing positive terms); the rest on DVE.  This
                    # balances the two engines at ~10 us/iter each.
                    junk = wp.tile([128, PCH], f32, tag="junk", bufs=2, name="junk")
                    if (n * NPCH + j) % 4 == 3 or (n * NPCH + j) == 17:
                        imm = lambda v: mybir.ImmediateValue(
                            dtype=mybir.dt.float32, value=v
                        )
                        eng = nc.scalar
                        eng.add_instruction(
                            mybir.InstActivation(
                                name=nc.get_next_instruction_name(),
                                func=Act.Reciprocal,
                                ins=[
                                    eng.lower_ap(q3[:, :]),
                                    imm(0.0), imm(1.0), imm(0.0),
                                ],
                                outs=[
                                    eng.lower_ap(junk[:, :]),
                                    eng.lower_ap(Racc[:, n, j : j + 1]),
                                ],
                            )
                        )
                    else:
                        r = wp.tile([128, PCH], f32, tag="r", bufs=2, name="r")
                        nc.vector.reciprocal_approx_fast(out=r[:, :], in_=q3[:, :])
                        nc.vector.tensor_scalar(
                            junk[:, :], r[:, :], 0.0, None, Alu.add, Alu.add,
                            accum_out=Racc[:, n, j : j + 1],
                        )
                    # warm the PE p-state shortly before each half's matmuls
                    if n % (NBLK // 2) >= NBLK // 2 - 2:
                        psd = psp.tile([128, NOUT], f32, tag="ps", name="psd")
                        nc.tensor.matmul(
                            psd[:, :],
                            xv_sb[0:1, 0:128],
                            Bt[0][0:1, 0:NOUT],
                            start=True,
                            stop=True,
                        )

            HB = NBLK // 2          # blocks per epilogue half
            HC = HB * 128           # c-columns per half

            def epilogue(half):
                nsl = slice(half * HB, (half + 1) * HB)
                Rsum = pp.tile([128, HB], f32, name="Rsum", tag="Rsum", bufs=2)
                nc.vector.tensor_tensor(
                    Rsum[:, :], Racc[:, nsl, 0], Racc[:, nsl, 1], Alu.add
                )
                nc.vector.tensor_tensor(
                    Rsum[:, :], Rsum[:, :], ps2_sb[:, nsl], Alu.mult
                )
                coef = pp.tile([128, HB], f32, name="coef", tag="coef", bufs=2)
                nc.vector.reciprocal(coef[:, :], Rsum[:, :])

                # transpose coef (128, HB) -> row (1, HC) via a DRAM bounce
                nc.sync.dma_start(
                    out=scr.rearrange("(p n) -> p n", p=128)[:, nsl], in_=coef[:, :]
                )
                crow = pp.tile([1, HC], f32, name="crow", tag="crow", bufs=2)
                nc.sync.dma_start(
                    out=crow[0:1, :].rearrange("a (n p) -> a n p", n=HB),
                    in_=scr.rearrange("(p n) -> n p", n=NBLK)[nsl, :],
                )

                # y[ch, c] = x[ch] * coef[c] as K=1 outer-product matmuls
                for h in range(CH // 128):
                    for qk in range(HC // NOUT):
                        ps = psp.tile([128, NOUT], f32, tag="ps", name="ps")
                        nc.tensor.matmul(
                            ps[:, :],
                            xv_sb[0:1, h * 128 : (h + 1) * 128],
                            crow[0:1, qk * NOUT : (qk + 1) * NOUT],
                            start=True,
                            stop=True,
                        )
                        ysb = wp.tile([128, NOUT], f32, tag="ysb", bufs=2, name="ysb")
                        nc.scalar.copy(ysb[:, :], ps[:, :])
                        nc.sync.dma_start(
                            out=y[
                                h * 128 : (h + 1) * 128,
                                half * HC + qk * NOUT : half * HC + (qk + 1) * NOUT,
                            ],
                            in_=ysb[:, :],
                        )

            def whole():
                bcast_loop()
                main_loop(0, NBLK // 2)
                epilogue(0)
                main_loop(NBLK // 2, NBLK)
                epilogue(1)

            if bench_nrep is None:
                whole()
            elif bench_span == "main":
                bcast_loop()
                with tc.For_i(0, bench_nrep, 1):
                    main_loop(0, NBLK)
                epilogue(0)
                epilogue(1)
            elif bench_span == "bcast":
                with tc.For_i(0, bench_nrep, 1):
                    bcast_loop()
                main_loop(0, NBLK)
                epilogue(0)
                epilogue(1)
            elif bench_span == "epi":
                bcast_loop()
                main_loop(0, NBLK)
                with tc.For_i(0, bench_nrep, 1):
                    epilogue(0)
                    epilogue(1)
            else:
                import concourse.mybir as _mb

                with tc.For_i(
                    0, bench_nrep, 1,
                    staggered_reset=True,
                    hint_engines=(_mb.EngineType.DVE, _mb.EngineType.Activation),
                ):
                    whole()
    nc.finalize()
    return nc


def _get_nc():
    if "nc" not in _cache:
        _cache["nc"] = _build()
    return _cache["nc"]


def _in_maps(x, mu, sig):
    maps = []
    for k in range(NCORES):
        b = k // 2
        half = k % 2
        sl = slice(half * CW, (half + 1) * CW)
        mu_b = np.asarray(mu[b], dtype=np.float32)
        sig_c = np.asarray(sig[b, sl], dtype=np.float32)
        inv = (sig_c * sig_c).astype(np.float32)          # s2
        nbs = (-mu_b[sl]).astype(np.float32)              # -mu
        ps2 = inv.reshape(NBLK, 128, 4).prod(axis=2, dtype=np.float32)

        def _rearr(a):
            return np.ascontiguousarray(
                a.reshape(NBLK, 128, D).transpose(1, 0, 2).reshape(128, -1)
            )

        maps.append(
            {
                "ptsT": np.ascontiguousarray(mu_b.T),
                "isg_r": _rearr(inv),
                "nbs_r": _rearr(nbs),
                "ps2_r": np.ascontiguousarray(ps2.T),
                "xv": np.ascontiguousarray(
                    np.asarray(x[b, :, 0], dtype=np.float32)[None, :]
                ),
            }
        )
    return maps


def kernel(x, pi, mu, sig):
    from concourse.bass_utils import run_bass_kernel_spmd

    nc = _get_nc()
    res = run_bass_kernel_spmd(nc, _in_maps(x, mu, sig), list(range(NCORES))).results
    y = np.empty((B, CH, C), np.float32)
    for k in range(NCORES):
        b = k // 2
        half = k % 2
        y[b, :, half * CW : (half + 1) * CW] = res[k]["y"]
    return y



# revision 4
# speedup vs baseline: 1.8051x; 1.8051x over previous
"""Trainium2 Bass kernel for nn_MixtureAttention.

Math: the reference builds a (c,c) pairwise Cauchy-product matrix per batch,
row-normalizes it, and keeps only the diagonal.  With
    qn(i,j) = prod_d (1 + (mu[j,d]-mu[i,d])^2 / sig[i,d]^2)
the kept diagonal reduces to   coef[i] = 1 / sum_j 1/qn(i,j)
(`pi` cancels in the row normalization), and y[b,ch,c] = x[b,ch] * coef[b,c].

Kernel: qn = q01 * q23 where each pair-of-dims factor is a degree-(2,2)
polynomial in the point coordinates -> a K=9 feature matmul per pair.
Each fp32 feature is split into three fp16 parts (hi/mid/lo); the six
product combinations hh,hm,mh,hl,mm,lh are stacked along the contraction
dim (K=54) so ONE fp16 matmul per pair computes the full product to
~5e-10 relative (dropped terms ~2^-33).  The two pair matmuls run
row-tiled (contraction bases 0 and 64) so they overlap in the PE array.
Per (128-row, 1024-point) tile: ACT computes u01 = 1/q01 (raw Reciprocal,
~1.2e-5) PSUM->SBUF, DVE computes recip1NR(q23)*u01 with accumulated
row-sum in one fused custom op (~0.17% max, equioscillating).  The
epilogue forms coef = 1/S, converts to fp16, transposes via a DRAM
bounce, and emits y = x (x) coef as K=1 fp16 outer-product matmuls; it
runs in two halves overlapped with the main loop.

Sharding: 8 cores; core k handles batch k//2, c-rows [(k%2)*2048, +2048).
"""

import numpy as np

B, C, D, CH = 4, 4096, 4, 256
NCORES = 8
CW = C // 2            # 2048 c-rows per core (2 cores per batch)
NBLK = CW // 128       # 16 row blocks
GW = 1024              # point-group width (2 PSUM banks per pair factor)
NG = C // GW           # 4 groups per row block
KS = 54                # stacked contraction dim (6 fp16-split combos x K=9)

_cache = {}


def _register_op(name, spec):
    """Register a custom DVE op into concourse's op table at runtime; uop
    shas are self-pinned by compiling once and reading the reported digest."""
    import re

    from concourse import dve_ops as DO

    key = "op_" + name
    if key in _cache:
        return _cache[key]
    shas = {}
    for ver in ("v3", "v4"):
        probe = DO.DveOp(name + "_PROBE", spec, subdim=False, uops_sha={})
        if name + "_PROBE" not in DO._SUB_OPCODE_FOR_NAME:
            DO._SUB_OPCODE_FOR_NAME[name + "_PROBE"] = 0x1F
        try:
            probe.compile(ver)
        except ValueError as e:
            m = re.search(r'"(?:v3|v4)"\]="([0-9a-f]+)"', str(e))
            if not m:
                raise
            shas[ver] = m.group(1)
    op = DO.DveOp(name, spec, subdim=False, uops_sha=shas)
    if name not in DO._SUB_OPCODE_FOR_NAME:
        DO.OPS.append(op)
        DO._SUB_OPCODE_FOR_NAME[name] = DO._CUSTOM_DVE_ROW_BASE + len(DO.OPS) - 1
        assert DO._SUB_OPCODE_FOR_NAME[name] < 0x20, "opcode rows exhausted"
    DO.CUSTOM_DVE_SPECS[name] = spec
    _cache[key] = op
    return op


def _np_nr1(x, c0, c1):
    nx = (~x.view(np.int32)).view(np.float32)
    y0 = (nx * np.float32(c0)).astype(np.float32)
    return (y0 * (np.float32(c1) - x * y0)).astype(np.float32)


# Chebyshev pair for the 1-NR fast reciprocal (same interval as concourse's
# RECIPROCAL_APPROX_FAST; one NR step -> ~1.7e-3 max, sign-balanced).
RC0, RC1 = -0.23549792, 2.0017324


def _get_rmacc():
    """out = recip1NR(Src0) * Src1, accum_out = row-sum(out).  7 DVE stages."""
    import operator

    from concourse.dve_spec import C0, C1, Bin, Spec, Src0, Src1, Zero
    from concourse.dve_uop import AluOp

    nx = Bin(AluOp.BITWISE_NOT, Src0, Src0)
    y0 = nx * C0
    y1 = y0 * (C1 - Src0 * y0)

    def _ref(in0, in1, c0, c1, c2):
        b = (_np_nr1(in0, c0, c1) * in1).astype(np.float32)
        return b, b.reshape(b.shape[0], -1).sum(axis=-1, keepdims=True)

    return _register_op(
        "RECIP1_MUL_ACC_ANT",
        Spec(body=y1 * Src1, accum=operator.add, accum_init=Zero, reference=_ref),
    )


def _build(bench_nrep=None, bench_span="full"):
    import concourse.bacc as bacc
    import concourse.mybir as mybir
    from concourse.tile import TileContext

    f32 = mybir.dt.float32
    f16 = mybir.dt.float16
    Act = mybir.ActivationFunctionType

    rmacc = _get_rmacc()
    nc = bacc.Bacc(None, target_bir_lowering=False)
    af = nc.declare_dram_parameter("af", [118, CW], f16, isOutput=False)
    pf = nc.declare_dram_parameter("pf", [118, C], f16, isOutput=False)
    xv = nc.declare_dram_parameter("xv", [1, CH], f16, isOutput=False)
    y = nc.declare_dram_parameter("y", [CH, CW], f32, isOutput=True)

    imm = lambda v: mybir.ImmediateValue(dtype=f32, value=v)

    with TileContext(nc) as tc:
        with (
            tc.tile_pool(name="persist", bufs=1) as pp,
            tc.tile_pool(name="work", bufs=1) as wp,
            tc.tile_pool(name="psum", bufs=2, space="PSUM") as psp,
            tc.tile_pool(name="dram", bufs=1, space="DRAM") as dp,
        ):
            scr = dp.tile([CW], f16, name="scr")
            afs = pp.tile([118, CW], f16)
            pfs = pp.tile([118, C], f16)
            xvs = pp.tile([1, CH], f16)

            def load_inputs():
                nc.sync.dma_start(out=afs[:, :], in_=af[:, :])
                nc.sync.dma_start(out=pfs[:, :], in_=pf[:, :])
                nc.sync.dma_start(out=xvs[0:1, :], in_=xv[0:1, :])

            Racc = pp.tile([128, NBLK, NG], f32)

            def main_loop(n_lo, n_hi):
                for n in range(n_lo, n_hi):
                    csl = slice(n * 128, (n + 1) * 128)
                    for g in range(NG):
                        p01 = psp.tile([128, 2, 512], f32, tag="p01", name="p01")
                        p23 = psp.tile([128, 2, 512], f32, tag="p23", name="p23",
                                       bufs=1)
                        for q in range(2):
                            sl = slice(g * GW + q * 512, g * GW + (q + 1) * 512)
                            nc.tensor.matmul(
                                p01[:, q, :], afs[0:KS, csl], pfs[0:KS, sl],
                                start=True, stop=True,
                            )
                            nc.tensor.matmul(
                                p23[:, q, :], afs[64:64 + KS, csl],
                                pfs[64:64 + KS, sl],
                                start=True, stop=True,
                            )
                        u01 = wp.tile([128, 2, 512], f32, tag="u01", bufs=3,
                                      name="u01")
                        eng = nc.scalar
                        eng.add_instruction(
                            mybir.InstActivation(
                                name=nc.get_next_instruction_name(),
                                func=Act.Reciprocal,
                                ins=[eng.lower_ap(p01[:, :, :]), imm(0.0),
                                     imm(1.0), imm(0.0)],
                                outs=[eng.lower_ap(u01[:, :, :])],
                            )
                        )
                        junk = wp.tile([128, 2, 512], f32, tag="junk", bufs=2,
                                       name="junk")
                        nc.vector._custom_dve(
                            rmacc, out=junk[:, :, :], in0=p23[:, :, :],
                            in1=u01[:, :, :], s0=RC0, s1=RC1,
                            accum_out=Racc[:, n, g:g + 1],
                        )

            HB = NBLK // 2          # row blocks per epilogue half
            HC = HB * 128           # c-columns per half

            def epilogue(half):
                nsl = slice(half * HB, (half + 1) * HB)
                Rsum = wp.tile([128, HB], f32, name="Rsum", tag="Rsum", bufs=2)
                nc.vector.tensor_tensor(
                    Rsum[:, :], Racc[:, nsl, 0], Racc[:, nsl, 1],
                    mybir.AluOpType.add,
                )
                nc.vector.tensor_tensor(
                    Rsum[:, :], Rsum[:, :], Racc[:, nsl, 2],
                    mybir.AluOpType.add,
                )
                nc.vector.tensor_tensor(
                    Rsum[:, :], Rsum[:, :], Racc[:, nsl, 3],
                    mybir.AluOpType.add,
                )
                coef = wp.tile([128, HB], f16, name="coef", tag="coef", bufs=2)
                eng = nc.scalar
                eng.add_instruction(
                    mybir.InstActivation(
                        name=nc.get_next_instruction_name(),
                        func=Act.Reciprocal,
                        ins=[eng.lower_ap(Rsum[:, :]), imm(0.0), imm(1.0),
                             imm(0.0)],
                        outs=[eng.lower_ap(coef[:, :])],
                    )
                )
                # transpose coef (128, HB) -> row (1, HC) via a DRAM bounce
                nc.sync.dma_start(
                    out=scr.rearrange("(p n) -> p n", p=128)[:, nsl],
                    in_=coef[:, :],
                )
                crow = wp.tile([1, HC], f16, name="crow", tag="crow", bufs=2)
                nc.sync.dma_start(
                    out=crow[0:1, :].rearrange("a (n p) -> a n p", n=HB),
                    in_=scr.rearrange("(p n) -> n p", n=NBLK)[nsl, :],
                )
                # y[ch, c] = x[ch] * coef[c] as K=1 fp16 outer-product matmuls
                for h in range(CH // 128):
                    for qk in range(HC // 512):
                        ps = psp.tile([128, 512], f32, tag="ps", name="ps")
                        nc.tensor.matmul(
                            ps[:, :],
                            xvs[0:1, h * 128:(h + 1) * 128],
                            crow[0:1, qk * 512:(qk + 1) * 512],
                            start=True, stop=True,
                        )
                        ysb = wp.tile([128, 512], f32, tag="ysb", bufs=2,
                                      name="ysb")
                        nc.scalar.copy(ysb[:, :], ps[:, :])
                        nc.sync.dma_start(
                            out=y[h * 128:(h + 1) * 128,
                                  half * HC + qk * 512:half * HC + (qk + 1) * 512],
                            in_=ysb[:, :],
                        )

            def whole():
                load_inputs()
                main_loop(0, NBLK // 2)
                epilogue(0)
                main_loop(NBLK // 2, NBLK)
                epilogue(1)

            if bench_nrep is None:
                whole()
            else:
                import concourse.mybir as _mb

                with tc.For_i(
                    0, bench_nrep, 1,
                    staggered_reset=True,
                    hint_engines=(_mb.EngineType.DVE, _mb.EngineType.Activation),
                ):
                    whole()
    nc.finalize()
    return nc


def _get_nc():
    if "nc" not in _cache:
        _cache["nc"] = _build()
    return _cache["nc"]


def _split3_f16(X):
    h = X.astype(np.float16)
    m = (X - h.astype(np.float64)).astype(np.float16)
    l = (X - h.astype(np.float64) - m.astype(np.float64)).astype(np.float16)
    return h, m, l


def _pair_features(mu_b, sig_b, rows, dims):
    """Stacked-K54 fp16 features: A [54, len(rows)], P [54, C]."""
    import itertools

    m = mu_b.astype(np.float64) - 0.5
    s2 = sig_b.astype(np.float64) ** 2
    cs = np.stack([(m * m + s2) / s2, -2 * m / s2, 1.0 / s2], axis=2)  # (C,D,3)
    fs = np.stack([np.ones_like(m), m, m * m], axis=2)                 # (C,D,3)
    d0, d1 = dims
    A = np.empty((9, len(rows)))
    P = np.empty((9, C))
    for k, (e0, e1) in enumerate(itertools.product(range(3), repeat=2)):
        A[k] = cs[rows, d0, e0] * cs[rows, d1, e1]
        P[k] = fs[:, d0, e0] * fs[:, d1, e1]
    # per-feature scale balancing keeps both sides in fp16 range and the
    # lo parts clear of subnormals
    s = np.sqrt(np.abs(P).max(axis=1) / np.abs(A).max(axis=1))
    A *= s[:, None]
    P /= s[:, None]
    Ah, Am, Al = _split3_f16(A)
    Ph, Pm, Pl = _split3_f16(P)
    # kept fp16-split products: hh, hm, mh, hl, mm, lh
    As = np.concatenate([Ah, Ah, Am, Ah, Am, Al], axis=0)
    Ps = np.concatenate([Ph, Pm, Ph, Pl, Pm, Ph], axis=0)
    return As, Ps


def _in_maps(x, mu, sig):
    maps = []
    for k in range(NCORES):
        b = k // 2
        half = k % 2
        rows = np.arange(half * CW, (half + 1) * CW)
        A01, P01 = _pair_features(mu[b], sig[b], rows, (0, 1))
        A23, P23 = _pair_features(mu[b], sig[b], rows, (2, 3))
        af = np.zeros((118, CW), np.float16)
        af[0:KS] = A01
        af[64:64 + KS] = A23
        pf = np.zeros((118, C), np.float16)
        pf[0:KS] = P01
        pf[64:64 + KS] = P23
        maps.append(
            {
                "af": af,
                "pf": pf,
                "xv": np.ascontiguousarray(
                    np.asarray(x[b, :, 0], dtype=np.float16)[None, :]
                ),
            }
        )
    return maps


def kernel(x, pi, mu, sig):
    from concourse.bass_utils import run_bass_kernel_spmd

    nc = _get_nc()
    res = run_bass_kernel_spmd(nc, _in_maps(x, mu, sig), list(range(NCORES))).results
    y = np.empty((B, CH, C), np.float32)
    for k in range(NCORES):
        b = k // 2
        half = k % 2
        y[b, :, half * CW:(half + 1) * CW] = res[k]["y"]
    return y


# revision 9
# speedup vs baseline: 2.4405x; 1.3520x over previous
"""Trainium2 Bass kernel for nn_MixtureAttention.

Math: the reference builds a (c,c) pairwise Cauchy-product matrix per batch,
row-normalizes it, and keeps only the diagonal.  With
    qn(i,j) = prod_d (1 + (mu[j,d]-mu[i,d])^2 / sig[i,d]^2)
the kept diagonal reduces to   coef[i] = 1 / sum_j 1/qn(i,j)
(`pi` cancels in the row normalization), and y[b,ch,c] = x[b,ch] * coef[b,c].

Kernel: qn = q01 * q23 where each pair-of-dims factor is a degree-(2,2)
polynomial in the point coordinates -> a K=9 feature matmul per pair.
Each fp32 feature is split into three fp16 parts (hi/mid/lo); the six
product combinations hh,hm,mh,hl,mm,lh are stacked along the contraction
dim (K=54) so ONE fp16 matmul per pair computes the full product to
~5e-10 relative (dropped terms ~2^-33).  The two pair matmuls run
row-tiled (contraction bases 0 and 64) so they overlap in the PE array.
Per (128-row, 1024-point) tile: ACT computes u01 = 1/q01 (raw Reciprocal,
~1.2e-5) PSUM->SBUF, DVE computes recip1NR(q23)*u01 with accumulated
row-sum in one fused custom op (~0.17% max, equioscillating).  The
epilogue forms coef = 1/S, converts to fp16, transposes via a DRAM
bounce, and emits y = x (x) coef as K=1 fp16 outer-product matmuls; it
runs in two halves overlapped with the main loop.

Sharding: 8 cores; core k handles batch k//2, c-rows [(k%2)*2048, +2048).
"""

import numpy as np

B, C, D, CH = 4, 4096, 4, 256
NCORES = 8
CW = C // 2            # 2048 c-rows per core (2 cores per batch)
NBLK = CW // 128       # 16 row blocks
GW = 1024              # point-group width (2 PSUM banks per pair factor)
NG = C // GW           # 4 groups per row block
KS = 54                # stacked contraction dim (6 fp16-split combos x K=9)

_cache = {}


def _register_op(name, spec):
    """Register a custom DVE op into concourse's op table at runtime; uop
    shas are self-pinned by compiling once and reading the reported digest."""
    import re

    from concourse import dve_ops as DO

    key = "op_" + name
    if key in _cache:
        return _cache[key]
    shas = {}
    for ver in ("v3", "v4"):
        probe = DO.DveOp(name + "_PROBE", spec, subdim=False, uops_sha={})
        if name + "_PROBE" not in DO._SUB_OPCODE_FOR_NAME:
            DO._SUB_OPCODE_FOR_NAME[name + "_PROBE"] = 0x1F
        try:
            probe.compile(ver)
        except ValueError as e:
            m = re.search(r'"(?:v3|v4)"\]="([0-9a-f]+)"', str(e))
            if not m:
                raise
            shas[ver] = m.group(1)
    op = DO.DveOp(name, spec, subdim=False, uops_sha=shas)
    if name not in DO._SUB_OPCODE_FOR_NAME:
        DO.OPS.append(op)
        DO._SUB_OPCODE_FOR_NAME[name] = DO._CUSTOM_DVE_ROW_BASE + len(DO.OPS) - 1
        assert DO._SUB_OPCODE_FOR_NAME[name] < 0x20, "opcode rows exhausted"
    DO.CUSTOM_DVE_SPECS[name] = spec
    _cache[key] = op
    return op


def _np_nr1(x, c0, c1):
    nx = (~x.view(np.int32)).view(np.float32)
    y0 = (nx * np.float32(c0)).astype(np.float32)
    return (y0 * (np.float32(c1) - x * y0)).astype(np.float32)


# Chebyshev pair for the 1-NR fast reciprocal (same interval as concourse's
# RECIPROCAL_APPROX_FAST; one NR step -> ~1.7e-3 max, sign-balanced).
RC0, RC1 = -0.23549792, 2.0017324


def _get_rmacc():
    """out = recip1NR(Src0) * Src1, accum_out = row-sum(out).  7 DVE stages."""
    import operator

    from concourse.dve_spec import C0, C1, Bin, Spec, Src0, Src1, Zero
    from concourse.dve_uop import AluOp

    nx = Bin(AluOp.BITWISE_NOT, Src0, Src0)
    y0 = nx * C0
    y1 = y0 * (C1 - Src0 * y0)

    def _ref(in0, in1, c0, c1, c2):
        b = (_np_nr1(in0, c0, c1) * in1).astype(np.float32)
        return b, b.reshape(b.shape[0], -1).sum(axis=-1, keepdims=True)

    return _register_op(
        "RECIP1_MUL_ACC_ANT",
        Spec(body=y1 * Src1, accum=operator.add, accum_init=Zero, reference=_ref),
    )


def _build(bench_nrep=None, bench_span="full"):
    import concourse.bacc as bacc
    import concourse.mybir as mybir
    from concourse.tile import TileContext

    f32 = mybir.dt.float32
    f16 = mybir.dt.float16
    Act = mybir.ActivationFunctionType

    rmacc = _get_rmacc()
    nc = bacc.Bacc(None, target_bir_lowering=False)
    af = nc.declare_dram_parameter("af", [118, CW], f16, isOutput=False)
    pf = nc.declare_dram_parameter("pf", [118, C], f16, isOutput=False)
    xv = nc.declare_dram_parameter("xv", [1, CH], f16, isOutput=False)
    y = nc.declare_dram_parameter("y", [CH, CW], f32, isOutput=True)

    imm = lambda v: mybir.ImmediateValue(dtype=f32, value=v)

    with TileContext(nc) as tc:
        with (
            tc.tile_pool(name="persist", bufs=1) as pp,
            tc.tile_pool(name="work", bufs=1) as wp,
            tc.tile_pool(name="psum", bufs=2, space="PSUM") as psp,
            tc.tile_pool(name="dram", bufs=1, space="DRAM") as dp,
        ):
            scr = dp.tile([CW], f16, name="scr")
            afs = pp.tile([118, CW], f16)
            pfs = pp.tile([118, C], f16)
            xvs = pp.tile([1, CH], f16)

            def load_inputs():
                nc.sync.dma_start(out=afs[:, :], in_=af[:, :])
                nc.sync.dma_start(out=pfs[:, :], in_=pf[:, :])
                nc.sync.dma_start(out=xvs[0:1, :], in_=xv[0:1, :])

            Racc = pp.tile([128, NBLK, NG], f32)

            u01_const = pp.tile([128, 2, 512], f32, name="u01c")

            def main_loop(n_lo, n_hi, mode="all"):
                for n in range(n_lo, n_hi):
                    csl = slice(n * 128, (n + 1) * 128)
                    for g in range(NG):
                        p01 = psp.tile([128, 2, 512], f32, tag="p01", name="p01")
                        p23 = psp.tile([128, 2, 512], f32, tag="p23", name="p23")
                        for q in range(2):
                            sl = slice(g * GW + q * 512, g * GW + (q + 1) * 512)
                            nc.tensor.matmul(
                                p23[:, q, :], afs[64:64 + KS, csl],
                                pfs[64:64 + KS, sl],
                                start=True, stop=True,
                            )
                        for q in range(2):
                            sl = slice(g * GW + q * 512, g * GW + (q + 1) * 512)
                            nc.tensor.matmul(
                                p01[:, q, :], afs[0:KS, csl], pfs[0:KS, sl],
                                start=True, stop=True,
                            )
                        eng = nc.scalar
                        if mode in ("all", "mmact"):
                            u01 = wp.tile([128, 2, 512], f32, tag="u01", bufs=3,
                                          name="u01")
                            eng.add_instruction(
                                mybir.InstActivation(
                                    name=nc.get_next_instruction_name(),
                                    func=Act.Reciprocal,
                                    ins=[eng.lower_ap(p01[:, :, :]), imm(0.0),
                                         imm(1.0), imm(0.0)],
                                    outs=[eng.lower_ap(u01[:, :, :])],
                                )
                            )
                        if mode in ("all", "mmdve"):
                            src1 = u01 if mode == "all" else u01_const
                            junk = wp.tile([128, 2, 512], f32, tag="junk", bufs=2,
                                           name="junk")
                            nc.vector._custom_dve(
                                rmacc, out=junk[:, :, :], in0=p23[:, :, :],
                                in1=src1[:, :, :], s0=RC0, s1=RC1,
                                accum_out=Racc[:, n, g:g + 1],
                            )

            HB = NBLK // 2          # row blocks per epilogue half
            HC = HB * 128           # c-columns per half

            def epilogue(half):
                nsl = slice(half * HB, (half + 1) * HB)
                Rsum = wp.tile([128, HB], f32, name="Rsum", tag="Rsum", bufs=2)
                nc.vector.tensor_tensor(
                    Rsum[:, :], Racc[:, nsl, 0], Racc[:, nsl, 1],
                    mybir.AluOpType.add,
                )
                nc.vector.tensor_tensor(
                    Rsum[:, :], Rsum[:, :], Racc[:, nsl, 2],
                    mybir.AluOpType.add,
                )
                nc.vector.tensor_tensor(
                    Rsum[:, :], Rsum[:, :], Racc[:, nsl, 3],
                    mybir.AluOpType.add,
                )
                coef = wp.tile([128, HB], f16, name="coef", tag="coef", bufs=2)
                eng = nc.scalar
                eng.add_instruction(
                    mybir.InstActivation(
                        name=nc.get_next_instruction_name(),
                        func=Act.Reciprocal,
                        ins=[eng.lower_ap(Rsum[:, :]), imm(0.0), imm(1.0),
                             imm(0.0)],
                        outs=[eng.lower_ap(coef[:, :])],
                    )
                )
                # transpose coef (128, HB) -> row (1, HC) via a DRAM bounce
                nc.sync.dma_start(
                    out=scr.rearrange("(p n) -> p n", p=128)[:, nsl],
                    in_=coef[:, :],
                )
                crow = wp.tile([1, HC], f16, name="crow", tag="crow", bufs=2)
                nc.sync.dma_start(
                    out=crow[0:1, :].rearrange("a (n p) -> a n p", n=HB),
                    in_=scr.rearrange("(p n) -> n p", n=NBLK)[nsl, :],
                )
                # y[ch, c] = x[ch] * coef[c] as K=1 fp16 outer-product matmuls
                for h in range(CH // 128):
                    ps = psp.tile([128, 2, 512], f32, tag="p01", name="eps")
                    for qk in range(2):
                        nc.tensor.matmul(
                            ps[:, qk, :],
                            xvs[0:1, h * 128:(h + 1) * 128],
                            crow[0:1, qk * 512:(qk + 1) * 512],
                            start=True, stop=True,
                        )
                    ysb = wp.tile([128, 2, 512], f32, tag="ysb", bufs=2,
                                  name="ysb")
                    nc.scalar.copy(ysb[:, :, :], ps[:, :, :])
                    nc.sync.dma_start(
                        out=y[h * 128:(h + 1) * 128,
                              half * HC:half * HC + 1024].rearrange(
                                  "p (a b) -> p a b", a=2),
                        in_=ysb[:, :, :],
                    )

            def whole():
                load_inputs()
                main_loop(0, NBLK // 2)
                epilogue(0)
                main_loop(NBLK // 2, NBLK)
                epilogue(1)

            if bench_nrep is None:
                whole()
            elif bench_span == "full":
                import concourse.mybir as _mb

                with tc.For_i(
                    0, bench_nrep, 1,
                    staggered_reset=True,
                    hint_engines=(_mb.EngineType.DVE, _mb.EngineType.Activation),
                ):
                    whole()
            elif bench_span in ("mm", "mmact", "mmdve", "main"):
                load_inputs()
                nc.sync.dma_start(
                    out=u01_const[:, :, :],
                    in_=y[0:128, 0:1024].rearrange("p (a b) -> p a b", a=2),
                )
                with tc.For_i(0, bench_nrep, 1):
                    main_loop(0, NBLK, mode="all" if bench_span == "main"
                              else ("mm" if bench_span == "mm" else bench_span))
                if bench_span in ("main", "mmdve"):
                    epilogue(0)
                    epilogue(1)
            elif bench_span == "load":
                with tc.For_i(0, bench_nrep, 1):
                    load_inputs()
                main_loop(0, NBLK)
                epilogue(0)
                epilogue(1)
            elif bench_span == "epi":
                load_inputs()
                main_loop(0, NBLK)
                with tc.For_i(0, bench_nrep, 1):
                    epilogue(0)
                    epilogue(1)
    nc.finalize()
    return nc


def _get_nc():
    if "nc" not in _cache:
        _cache["nc"] = _build()
    return _cache["nc"]


def _split3_f16(X):
    h = X.astype(np.float16)
    m = (X - h.astype(np.float64)).astype(np.float16)
    l = (X - h.astype(np.float64) - m.astype(np.float64)).astype(np.float16)
    return h, m, l


def _pair_features(mu_b, sig_b, rows, dims):
    """Stacked-K54 fp16 features: A [54, len(rows)], P [54, C]."""
    import itertools

    m = mu_b.astype(np.float64) - 0.5
    s2 = sig_b.astype(np.float64) ** 2
    cs = np.stack([(m * m + s2) / s2, -2 * m / s2, 1.0 / s2], axis=2)  # (C,D,3)
    fs = np.stack([np.ones_like(m), m, m * m], axis=2)                 # (C,D,3)
    d0, d1 = dims
    A = np.empty((9, len(rows)))
    P = np.empty((9, C))
    for k, (e0, e1) in enumerate(itertools.product(range(3), repeat=2)):
        A[k] = cs[rows, d0, e0] * cs[rows, d1, e1]
        P[k] = fs[:, d0, e0] * fs[:, d1, e1]
    # per-feature scale balancing keeps both sides in fp16 range and the
    # lo parts clear of subnormals
    s = np.sqrt(np.abs(P).max(axis=1) / np.abs(A).max(axis=1))
    A *= s[:, None]
    P /= s[:, None]
    Ah, Am, Al = _split3_f16(A)
    Ph, Pm, Pl = _split3_f16(P)
    # kept fp16-split products: hh, hm, mh, hl, mm, lh
    As = np.concatenate([Ah, Ah, Am, Ah, Am, Al], axis=0)
    Ps = np.concatenate([Ph, Pm, Ph, Pl, Pm, Ph], axis=0)
    return As, Ps


def _in_maps(x, mu, sig):
    maps = []
    for k in range(NCORES):
        b = k // 2
        half = k % 2
        rows = np.arange(half * CW, (half + 1) * CW)
        A01, P01 = _pair_features(mu[b], sig[b], rows, (0, 1))
        A23, P23 = _pair_features(mu[b], sig[b], rows, (2, 3))
        af = np.zeros((118, CW), np.float16)
        af[0:KS] = A01
        af[64:64 + KS] = A23
        pf = np.zeros((118, C), np.float16)
        pf[0:KS] = P01
        pf[64:64 + KS] = P23
        maps.append(
            {
                "af": af,
                "pf": pf,
                "xv": np.ascontiguousarray(
                    np.asarray(x[b, :, 0], dtype=np.float16)[None, :]
                ),
            }
        )
    return maps


def kernel(x, pi, mu, sig):
    from concourse.bass_utils import run_bass_kernel_spmd

    nc = _get_nc()
    res = run_bass_kernel_spmd(nc, _in_maps(x, mu, sig), list(range(NCORES))).results
    y = np.empty((B, CH, C), np.float32)
    for k in range(NCORES):
        b = k // 2
        half = k % 2
        y[b, :, half * CW:(half + 1) * CW] = res[k]["y"]
    return y


# revision 11
# speedup vs baseline: 12.5955x; 5.1611x over previous
"""Trainium2 Bass kernel for nn_MixtureAttention.

Math: the reference builds a (c,c) pairwise Cauchy-product matrix per batch,
row-normalizes it, and keeps only the diagonal.  With
    qn(i,j) = prod_d (1 + (mu[j,d]-mu[i,d])^2 / sig[i,d]^2)
the kept diagonal reduces to   coef[i] = 1 / sum_j 1/qn(i,j)
(`pi` cancels in the row normalization), and y[b,ch,c] = x[b,ch] * coef[b,c].

Kernel: qn = q01 * q23 where each pair-of-dims factor is a degree-(2,2)
polynomial in the point coordinates -> a K=9 feature matmul per pair.
Each fp32 feature is split into three fp16 parts (hi/mid/lo); the six
product combinations hh,hm,mh,hl,mm,lh are stacked along the contraction
dim (K=54) so ONE fp16 matmul per pair computes the full product to
~5e-10 relative (dropped terms ~2^-33).  The two pair matmuls run
row-tiled (contraction bases 0 and 64) so they overlap in the PE array.
Per (128-row, 1024-point) tile: ACT computes u01 = 1/q01 (raw Reciprocal,
~1.2e-5) PSUM->SBUF, DVE computes recip1NR(q23)*u01 with accumulated
row-sum in one fused custom op (~0.17% max, equioscillating).  The
epilogue forms coef = 1/S, converts to fp16, transposes via a DRAM
bounce, and emits y = x (x) coef as K=1 fp16 outer-product matmuls; it
runs in two halves overlapped with the main loop.

Sharding: 8 cores; core k handles batch k//2, c-rows [(k%2)*2048, +2048).
"""

import numpy as np

B, C, D, CH = 4, 4096, 4, 256
NCORES = 8
CW = C // 2            # 2048 c-rows per core (2 cores per batch)
NBLK = CW // 128       # 16 row blocks
GW = 1024              # point-group width (2 PSUM banks per pair factor)
NG = C // GW           # 4 groups per row block
KS = 54                # stacked contraction dim (6 fp16-split combos x K=9)

_cache = {}


def _register_op(name, spec):
    """Register a custom DVE op into concourse's op table at runtime; uop
    shas are self-pinned by compiling once and reading the reported digest."""
    import re

    from concourse import dve_ops as DO

    key = "op_" + name
    if key in _cache:
        return _cache[key]
    shas = {}
    for ver in ("v3", "v4"):
        probe = DO.DveOp(name + "_PROBE", spec, subdim=False, uops_sha={})
        if name + "_PROBE" not in DO._SUB_OPCODE_FOR_NAME:
            DO._SUB_OPCODE_FOR_NAME[name + "_PROBE"] = 0x1F
        try:
            probe.compile(ver)
        except ValueError as e:
            m = re.search(r'"(?:v3|v4)"\]="([0-9a-f]+)"', str(e))
            if not m:
                raise
            shas[ver] = m.group(1)
    op = DO.DveOp(name, spec, subdim=False, uops_sha=shas)
    if name not in DO._SUB_OPCODE_FOR_NAME:
        DO.OPS.append(op)
        DO._SUB_OPCODE_FOR_NAME[name] = DO._CUSTOM_DVE_ROW_BASE + len(DO.OPS) - 1
        assert DO._SUB_OPCODE_FOR_NAME[name] < 0x20, "opcode rows exhausted"
    DO.CUSTOM_DVE_SPECS[name] = spec
    _cache[key] = op
    return op


def _np_nr1(x, c0, c1):
    nx = (~x.view(np.int32)).view(np.float32)
    y0 = (nx * np.float32(c0)).astype(np.float32)
    return (y0 * (np.float32(c1) - x * y0)).astype(np.float32)


# Chebyshev pair for the 1-NR fast reciprocal (same interval as concourse's
# RECIPROCAL_APPROX_FAST; one NR step -> ~1.7e-3 max, sign-balanced).
RC0, RC1 = -0.23549792, 2.0017324


def _get_rmacc():
    """out = recip1NR(Src0) * Src1, accum_out = row-sum(out).  7 DVE stages."""
    import operator

    from concourse.dve_spec import C0, C1, Bin, Spec, Src0, Src1, Zero
    from concourse.dve_uop import AluOp

    nx = Bin(AluOp.BITWISE_NOT, Src0, Src0)
    y0 = nx * C0
    y1 = y0 * (C1 - Src0 * y0)

    def _ref(in0, in1, c0, c1, c2):
        b = (_np_nr1(in0, c0, c1) * in1).astype(np.float32)
        return b, b.reshape(b.shape[0], -1).sum(axis=-1, keepdims=True)

    return _register_op(
        "RECIP1_MUL_ACC_ANT",
        Spec(body=y1 * Src1, accum=operator.add, accum_init=Zero, reference=_ref),
    )


def _build(bench_nrep=None, bench_span="full"):
    import concourse.bacc as bacc
    import concourse.mybir as mybir
    from concourse.tile import TileContext

    f32 = mybir.dt.float32
    f16 = mybir.dt.float16
    Act = mybir.ActivationFunctionType

    rmacc = _get_rmacc()
    nc = bacc.Bacc(None, target_bir_lowering=False)
    af = nc.declare_dram_parameter("af", [118, CW], f16, isOutput=False)
    pf = nc.declare_dram_parameter("pf", [118, C], f16, isOutput=False)
    xv = nc.declare_dram_parameter("xv", [1, CH], f32, isOutput=False)
    y = nc.declare_dram_parameter("y", [CW, CH], f32, isOutput=True)

    imm = lambda v: mybir.ImmediateValue(dtype=f32, value=v)

    with TileContext(nc) as tc:
        with (
            tc.tile_pool(name="persist", bufs=1) as pp,
            tc.tile_pool(name="work", bufs=1) as wp,
            tc.tile_pool(name="psum", bufs=2, space="PSUM") as psp,
            tc.tile_pool(name="dram", bufs=1, space="DRAM") as dp,
        ):
            afs = pp.tile([118, CW], f16)
            pfs = pp.tile([118, C], f16)
            xbc = pp.tile([128, CH], f32)

            def load_inputs():
                for i in range(4):
                    nc.sync.dma_start(
                        out=pfs[:, i * 1024:(i + 1) * 1024],
                        in_=pf[:, i * 1024:(i + 1) * 1024])
                for i in range(2):
                    nc.sync.dma_start(
                        out=afs[:, i * 1024:(i + 1) * 1024],
                        in_=af[:, i * 1024:(i + 1) * 1024])
                nc.sync.dma_start(
                    out=xbc[:, :], in_=xv[0:1, :].broadcast_to([128, CH]))

            Racc = pp.tile([128, NBLK, NG], f32)

            u01_const = pp.tile([128, 2, 512], f32, name="u01c")

            def main_loop(n_lo, n_hi, mode="all"):
                for n in range(n_lo, n_hi):
                    csl = slice(n * 128, (n + 1) * 128)
                    for g in range(NG):
                        p01 = psp.tile([128, 2, 512], f32, tag="p01", name="p01")
                        p23 = psp.tile([128, 2, 512], f32, tag="p23", name="p23")
                        for q in range(2):
                            sl = slice(g * GW + q * 512, g * GW + (q + 1) * 512)
                            nc.tensor.matmul(
                                p23[:, q, :], afs[64:64 + KS, csl],
                                pfs[64:64 + KS, sl],
                                start=True, stop=True,
                            )
                        for q in range(2):
                            sl = slice(g * GW + q * 512, g * GW + (q + 1) * 512)
                            nc.tensor.matmul(
                                p01[:, q, :], afs[0:KS, csl], pfs[0:KS, sl],
                                start=True, stop=True,
                            )
                        eng = nc.scalar
                        if mode in ("all", "mmact"):
                            u01 = wp.tile([128, 2, 512], f32, tag="u01", bufs=3,
                                          name="u01")
                            eng.add_instruction(
                                mybir.InstActivation(
                                    name=nc.get_next_instruction_name(),
                                    func=Act.Reciprocal,
                                    ins=[eng.lower_ap(p01[:, :, :]), imm(0.0),
                                         imm(1.0), imm(0.0)],
                                    outs=[eng.lower_ap(u01[:, :, :])],
                                )
                            )
                        if mode in ("all", "mmdve"):
                            src1 = u01 if mode == "all" else u01_const
                            junk = wp.tile([128, 2, 512], f32, tag="junk", bufs=2,
                                           name="junk")
                            nc.vector._custom_dve(
                                rmacc, out=junk[:, :, :], in0=p23[:, :, :],
                                in1=src1[:, :, :], s0=RC0, s1=RC1,
                                accum_out=Racc[:, n, g:g + 1],
                            )

            HB = NBLK // 2          # row blocks per epilogue half
            HC = HB * 128           # c-columns per half

            def epilogue(half):
                nsl = slice(half * HB, (half + 1) * HB)
                Rsum = wp.tile([128, HB], f32, name="Rsum", tag="Rsum", bufs=2)
                nc.vector.tensor_tensor(
                    Rsum[:, :], Racc[:, nsl, 0], Racc[:, nsl, 1],
                    mybir.AluOpType.add,
                )
                nc.vector.tensor_tensor(
                    Rsum[:, :], Rsum[:, :], Racc[:, nsl, 2],
                    mybir.AluOpType.add,
                )
                nc.vector.tensor_tensor(
                    Rsum[:, :], Rsum[:, :], Racc[:, nsl, 3],
                    mybir.AluOpType.add,
                )
                coef = wp.tile([128, HB], f32, name="coef", tag="coef", bufs=2)
                eng = nc.scalar
                eng.add_instruction(
                    mybir.InstActivation(
                        name=nc.get_next_instruction_name(),
                        func=Act.Reciprocal,
                        ins=[eng.lower_ap(Rsum[:, :]), imm(0.0), imm(1.0),
                             imm(0.0)],
                        outs=[eng.lower_ap(coef[:, :])],
                    )
                )
                # yT[c, ch] = coef[c] * x[ch]; per-partition scalar multiply,
                # contiguous DMA out (host transposes during reassembly)
                for j in range(HB):
                    n = half * HB + j
                    yt = wp.tile([128, CH], f32, tag="yt", bufs=4, name="yt")
                    nc.vector.tensor_scalar_mul(
                        yt[:, :], xbc[:, :], coef[:, j:j + 1],
                    )
                    nc.sync.dma_start(
                        out=y[n * 128:(n + 1) * 128, :], in_=yt[:, :],
                    )

            def whole():
                load_inputs()
                main_loop(0, NBLK // 2)
                epilogue(0)
                main_loop(NBLK // 2, NBLK)
                epilogue(1)

            if bench_nrep is None:
                whole()
            elif bench_span == "full":
                import concourse.mybir as _mb

                with tc.For_i(
                    0, bench_nrep, 1,
                    staggered_reset=True,
                    hint_engines=(_mb.EngineType.DVE, _mb.EngineType.Activation),
                ):
                    whole()
            elif bench_span in ("mm", "mmact", "mmdve", "main"):
                load_inputs()
                nc.sync.dma_start(
                    out=u01_const[:, :, :],
                    in_=y[0:128, 0:1024].rearrange("p (a b) -> p a b", a=2),
                )
                with tc.For_i(0, bench_nrep, 1):
                    main_loop(0, NBLK, mode="all" if bench_span == "main"
                              else ("mm" if bench_span == "mm" else bench_span))
                if bench_span in ("main", "mmdve"):
                    epilogue(0)
                    epilogue(1)
            elif bench_span == "load":
                with tc.For_i(0, bench_nrep, 1):
                    load_inputs()
                main_loop(0, NBLK)
                epilogue(0)
                epilogue(1)
            elif bench_span == "epi":
                load_inputs()
                main_loop(0, NBLK)
                with tc.For_i(0, bench_nrep, 1):
                    epilogue(0)
                    epilogue(1)
    nc.finalize()
    return nc


def _get_nc():
    if "nc" not in _cache:
        _cache["nc"] = _build()
    return _cache["nc"]


def _split3_f16(X):
    h = X.astype(np.float16)
    m = (X - h.astype(np.float64)).astype(np.float16)
    l = (X - h.astype(np.float64) - m.astype(np.float64)).astype(np.float16)
    return h, m, l


def _pair_features(mu_b, sig_b, rows, dims):
    """Stacked-K54 fp16 features: A [54, len(rows)], P [54, C]."""
    import itertools

    m = mu_b.astype(np.float64) - 0.5
    s2 = sig_b.astype(np.float64) ** 2
    cs = np.stack([(m * m + s2) / s2, -2 * m / s2, 1.0 / s2], axis=2)  # (C,D,3)
    fs = np.stack([np.ones_like(m), m, m * m], axis=2)                 # (C,D,3)
    d0, d1 = dims
    A = np.empty((9, len(rows)))
    P = np.empty((9, C))
    for k, (e0, e1) in enumerate(itertools.product(range(3), repeat=2)):
        A[k] = cs[rows, d0, e0] * cs[rows, d1, e1]
        P[k] = fs[:, d0, e0] * fs[:, d1, e1]
    # per-feature scale balancing keeps both sides in fp16 range and the
    # lo parts clear of subnormals
    s = np.sqrt(np.abs(P).max(axis=1) / np.abs(A).max(axis=1))
    A *= s[:, None]
    P /= s[:, None]
    Ah, Am, Al = _split3_f16(A)
    Ph, Pm, Pl = _split3_f16(P)
    # kept fp16-split products: hh, hm, mh, hl, mm, lh
    As = np.concatenate([Ah, Ah, Am, Ah, Am, Al], axis=0)
    Ps = np.concatenate([Ph, Pm, Ph, Pl, Pm, Ph], axis=0)
    return As, Ps


def _in_maps(x, mu, sig):
    maps = []
    for k in range(NCORES):
        b = k // 2
        half = k % 2
        rows = np.arange(half * CW, (half + 1) * CW)
        A01, P01 = _pair_features(mu[b], sig[b], rows, (0, 1))
        A23, P23 = _pair_features(mu[b], sig[b], rows, (2, 3))
        af = np.zeros((118, CW), np.float16)
        af[0:KS] = A01
        af[64:64 + KS] = A23
        pf = np.zeros((118, C), np.float16)
        pf[0:KS] = P01
        pf[64:64 + KS] = P23
        maps.append(
            {
                "af": af,
                "pf": pf,
                "xv": np.ascontiguousarray(
                    np.asarray(x[b, :, 0], dtype=np.float32)[None, :]
                ),
            }
        )
    return maps


def kernel(x, pi, mu, sig):
    from concourse.bass_utils import run_bass_kernel_spmd

    nc = _get_nc()
    res = run_bass_kernel_spmd(nc, _in_maps(x, mu, sig), list(range(NCORES))).results
    y = np.empty((B, CH, C), np.float32)
    for k in range(NCORES):
        b = k // 2
        half = k % 2
        y[b, :, half * CW:(half + 1) * CW] = res[k]["y"].T
    return y
